# revision 1
# baseline (speedup 1.0000x reference)
"""Trainium2 Bass kernel for nn_Encoder GNN message passing (8 NeuronCores).

Decomposition (dst-sharded, transposed compute):
  - nodes assigned to (core, sub-block<=32 nodes, pos) slots; each sub-block
    has <=512 in-edges, split into an A-stream (table rows < 32768) and a
    B-stream (rows >= 20480, indexed relative to 20480) of 2x128-edge chunks
    each, so indices fit dma_gather's int16.
  - per chunk: P^T[j, t*32+pos] += m_g[e, j]^T @ onehot[e, t*32+pos]
  - transform: agg^T[:, sub] = sum_t (2*W_t) @ P^T[:, t*32:(t+1)*32]
    (x2 because the reference duplicates every edge)
  - GRU (h == x0 always) computed transposed per 128-slot macro-block,
    gh recomputed per step from resident x0^T via PSUM accumulation.
  - m table [NSLOTS, 128] bf16 in DRAM, AllGather'd across cores per step.
"""
import sys
import types
import numpy as np
import ml_dtypes

import concourse.bass as bass
import concourse.mybir as mybir
import concourse.tile as tile
import concourse.tile_sem_assignment as _tsa
from concourse import library_config
from concourse.bass_utils import run_bass_kernel_spmd

# cap DMA completion-sem lanes so sync-wait splitting stays manageable
_tsa.NUM_SWDGE_GLOBAL_SEMS = 2
_tsa.NUM_HWDGE_SEMS = 2

N, IN, L, STEPS, T, E = 50000, 64, 128, 3, 4, 800000
NCORES = 8
SUB_CAP_N = 32
SUB_CAP_E = 512           # 4 chunks x 128
NODES_PER_CORE = N // NCORES
B_BASE = 20480            # B-stream table base row
A_MAX = 32768             # A-stream rows must be < 32768
PAD_COL = 200.0           # one-hot sentinel (never equals iota 0..127)

F32, BF16, I16 = mybir.dt.float32, mybir.dt.bfloat16, mybir.dt.int16
BF = ml_dtypes.bfloat16


# ---------------------------------------------------------------- waitfix --
def _fix_sync_waits(nc):
    """This walrus accepts only ONE semaphore wait per instruction; move
    excess waits onto preceding same-engine NoOps (engine queues are
    in-order, so semantics are preserved)."""
    uid = [0]

    def mknop(engine, waits, debug):
        uid[0] += 1
        return mybir.InstNoOp(
            name=f"WFIX-{uid[0]}", engine=engine, ins=[], outs=[], debug=debug,
            sync_info=mybir.SyncInfo(on_wait=list(waits), on_update=[]))

    total = 0
    for bb in nc.main_func.blocks:
        il = bb.instructions
        i = 0
        while i < len(il):
            inst = il[i]
            si = inst.sync_info
            waits = list(si.on_wait) if si is not None else []
            if len(waits) > 1:
                inst.sync_info = mybir.SyncInfo(
                    on_wait=waits[:1], on_update=list(si.on_update))
                nops = [mknop(inst.engine, [w], inst.debug)
                        for w in waits[1:]]
                for k, nop in enumerate(nops):
                    il.insert(i + k, nop)
                    nc.register_instruction(nop, overwrite=True)
                i += len(nops)
                total += len(nops)
            i += 1
    return total


def _install_ntff_hook():
    if "antenv.axon_hooks" in sys.modules:
        return
    try:
        from trn_agent_boot.trn_boot import _ntff_profile_via_ctypes
        hook = _ntff_profile_via_ctypes("/opt/axon/libaxon_pjrt.so")
    except Exception:
        hook = None
    mod = types.ModuleType("antenv.axon_hooks")
    mod.get_axon_ntff_profile_hook = lambda: hook
    mod.set_axon_ntff_profile_hook = lambda h: None
    sys.modules["antenv.axon_hooks"] = mod
    import concourse.bass_utils as bu
    bu.upload_artifacts = lambda d: f"local:{d}"


# ---------------------------------------------------------- preprocessing --
def _preprocess(edge_index, edge_attr):
    src = np.asarray(edge_index[0], np.int64)
    dst = np.asarray(edge_index[1], np.int64)
    bond = np.asarray(np.argmax(np.asarray(edge_attr), axis=1), np.int64)
    degsum = np.bincount(dst, minlength=N)

    # stage 1: nodes -> cores (balance total degree, exact node count)
    order = np.argsort(-degsum, kind="stable")
    node_core = np.full(N, -1, np.int64)
    core_load = np.zeros(NCORES, np.int64)
    core_count = np.zeros(NCORES, np.int64)
    for n in order:
        cand = np.flatnonzero(core_count < NODES_PER_CORE)
        c = cand[np.argmin(core_load[cand])]
        node_core[n] = c
        core_load[c] += degsum[n]
        core_count[c] += 1

    # stage 2: per-core packing into nsub bins (<=32 nodes, <=512 edges)
    nsub = max(int(np.ceil(NODES_PER_CORE / (SUB_CAP_N - 1.5))),
               int(np.ceil(core_load.max() / (SUB_CAP_E - 16))))
    nsub = -(-nsub // 16) * 16
    for _attempt in range(6):
        packs, ok = [], True
        for c in range(NCORES):
            nodes = np.flatnonzero(node_core == c)
            ds = degsum[nodes]
            node_sub = np.full(len(nodes), -1, np.int64)
            node_pos = np.full(len(nodes), -1, np.int64)
            cnt = np.zeros(nsub, np.int64)
            load = np.zeros(nsub, np.int64)
            soft_e, soft_n = SUB_CAP_E - 12, SUB_CAP_N - 1
            for i in np.argsort(-ds, kind="stable"):
                l2, c2 = load + ds[i], cnt + 1
                hard = (c2 > SUB_CAP_N) | (l2 > SUB_CAP_E)
                key = np.where(hard, 1e18,
                               np.maximum(l2 / soft_e, c2 / soft_n))
                b = int(np.argmin(key))
                if key[b] >= 1e17:
                    ok = False
                    break
                node_sub[i] = b
                node_pos[i] = cnt[b]
                cnt[b] += 1
                load[b] += ds[i]
            if not ok:
                break
            packs.append((nodes, node_sub, node_pos))
        if ok:
            break
        nsub += 16
    assert ok, "node packing failed"

    spc = nsub * 32                       # slots per core
    nslots = NCORES * spc
    node_slot = np.full(N, -1, np.int64)
    for c, (nodes, nsubv, nposv) in enumerate(packs):
        node_slot[nodes] = c * spc + nsubv * 32 + nposv

    # per-edge info
    e_core = node_core[dst]
    e_sub = (node_slot[dst] % spc) // 32
    e_col = bond * 32 + (node_slot[dst] % 32)
    e_srcslot = node_slot[src]

    # per-core chunk arrays in the group layout:
    #   group g = subs [16g, 16g+16); tile cols 0..31 = A chunks (2/sub),
    #   32..63 = B chunks (2/sub)
    ngroups = nsub // 16
    nchunks = nsub * 4
    # idx tensor: [128, ngroups*512] int16 (per group: 256 A cols + 256 B)
    gidx = np.zeros((NCORES, 128, ngroups * 512), np.int16)
    dstl = np.full((NCORES, 128, nchunks), PAD_COL, np.float32)

    for c in range(NCORES):
        esel = np.flatnonzero(e_core == c)
        sub = e_sub[esel]
        order_e = np.argsort(sub, kind="stable")
        es, subs_sorted = esel[order_e], sub[order_e]
        starts = np.searchsorted(subs_sorted, np.arange(nsub))
        ends = np.searchsorted(subs_sorted, np.arange(nsub), side="right")
        # linear index lists per group: A_lin[g][i] = table row, i -> (chunk
        # i//128 within the A half, partition i%128)
        A_lin = np.zeros((ngroups, 4096), np.int64)
        B_lin = np.zeros((ngroups, 4096), np.int64)
        for s in range(nsub):
            e_seg = es[starts[s]:ends[s]]
            slots = e_srcslot[e_seg]
            isA_must = slots < B_BASE
            isB_must = slots >= A_MAX
            band = ~(isA_must | isB_must)
            a_cnt = int(isA_must.sum())
            take_band_a = min(max(0, 256 - a_cnt), int(band.sum()))
            band_idx = np.flatnonzero(band)
            a_sel = np.concatenate(
                [np.flatnonzero(isA_must), band_idx[:take_band_a]])
            b_sel = np.concatenate(
                [np.flatnonzero(isB_must), band_idx[take_band_a:]])
            assert len(a_sel) <= 256 and len(b_sel) <= 256, \
                f"A/B split overflow sub {s}: {len(a_sel)} {len(b_sel)}"
            g, sl = s // 16, s % 16
            for sel, lin, base in ((a_sel, A_lin, 0), (b_sel, B_lin, B_BASE)):
                rows = slots[sel] - base
                cols = e_col[e_seg[sel]]
                k = np.arange(len(sel))
                lin[g, sl * 256 + k] = rows
                ch_off = (0 if base == 0 else 32) + 2 * sl
                dstl[c, k % 128, (g * 64 + ch_off + k // 128)] = cols
        # wrap indices: position i -> (partition i%16? no: (s p) wrap over 16)
        for g in range(ngroups):
            for half, lin in ((0, A_lin), (1, B_lin)):
                w = lin[g].reshape(256, 16).T        # [16, 256]
                gidx[c, :, g * 512 + half * 256:(g * 512 + half * 256) + 256] \
                    = np.tile(w, (8, 1))
    return dict(node_slot=node_slot, nsub=nsub, spc=spc, nslots=nslots,
                gidx=gidx, dstl=dstl)


# ------------------------------------------------------------- bass graph --
def _build(nsub, spc, nslots):
    nc = bass.Bass(target_bir_lowering=False, debug=False)
    ngroups = nsub // 16
    nmacro = nsub // 4
    nchunks = nsub * 4

    xT = nc.declare_dram_parameter("xT", [IN + 1, spc], F32, isOutput=False)
    lwT = nc.declare_dram_parameter("lwT", [IN + 1, 128], F32, isOutput=False)
    gidx = nc.declare_dram_parameter("gidx", [128, ngroups * 512], I16,
                                     isOutput=False)
    dstl = nc.declare_dram_parameter("dstl", [128, nchunks], BF16,
                                     isOutput=False)
    # consts bf16: iota(128) | identity(128) | w_ihT(384) | w_hhT(384) |
    #              WtT2 (12*128)
    CCOLS = 128 + 128 + 384 + 384 + 12 * 128
    consts = nc.declare_dram_parameter("consts", [128, CCOLS], BF16,
                                       isOutput=False)
    mulvT = nc.declare_dram_parameter("mulvT", [128, 256], F32, isOutput=False)
    biases = nc.declare_dram_parameter("biases", [128, 8], F32, isOutput=False)
    muo = nc.declare_dram_parameter("muo", [128, spc], F32, isOutput=True)
    lvo = nc.declare_dram_parameter("lvo", [128, spc], F32, isOutput=True)

    m_shard = nc.dram_tensor("m_shard", [spc, 128], BF16, kind="Internal")
    m_table = nc.dram_tensor("m_table", [nslots, 128], BF16, kind="Internal",
                             addr_space="Shared")

    with tile.TileContext(nc) as tc:
        with (
            tc.tile_pool(name="const", bufs=1) as cpool,
            tc.tile_pool(name="sb", bufs=2) as sb,
            tc.tile_pool(name="mg", bufs=2) as mgp,
            tc.tile_pool(name="oh", bufs=2) as ohp,
            tc.tile_pool(name="pt", bufs=2, space="PSUM") as ptp,
            tc.tile_pool(name="agg", bufs=1, space="PSUM") as aggp,
            tc.tile_pool(name="gate", bufs=1, space="PSUM") as gatep,
            tc.tile_pool(name="misc", bufs=1, space="PSUM") as miscp,
            tc.tile_pool(name="prep", bufs=2, space="PSUM") as prepp,
        ):
            nc.gpsimd.load_library(library_config.mlp)
            nidx_reg = nc.gpsimd.to_reg(4096)

            cst = cpool.tile([128, CCOLS], BF16)
            nc.sync.dma_start(cst[:], consts[:, :])
            iota = cst[:, 0:128]
            ident = cst[:, 128:256]
            wihT = cst[:, 256:640]
            whhT = cst[:, 640:1024]

            def WtT2(step, t):
                o = 1024 + (step * 4 + t) * 128
                return cst[:, o:o + 128]

            mulv_sb = cpool.tile([128, 256], F32)
            nc.sync.dma_start(mulv_sb[:], mulvT[:, :])
            bia = cpool.tile([128, 8], F32)
            nc.sync.dma_start(bia[:], biases[:, :])
            gidx_sb = cpool.tile([128, ngroups * 512], I16)
            nc.sync.dma_start(gidx_sb[:], gidx[:, :])
            dstl_sb = cpool.tile([128, nchunks], BF16)
            nc.sync.dma_start(dstl_sb[:], dstl[:, :])
            xT_sb = cpool.tile([IN + 1, spc], F32)
            nc.sync.dma_start(xT_sb[:], xT[:, :])
            lwT_sb = cpool.tile([IN + 1, 128], F32)
            nc.sync.dma_start(lwT_sb[:], lwT[:, :])

            # ---- x0 ----
            x0T_bf = cpool.tile([128, spc], BF16)
            x0T_f = cpool.tile([128, spc], F32)
            for i in range(0, spc, 512):
                w = min(512, spc - i)
                ps = prepp.tile([128, 512], F32, tag="pre")
                nc.tensor.matmul(out=ps[:, :w], lhsT=lwT_sb[:],
                                 rhs=xT_sb[:, i:i + w], start=True, stop=True)
                nc.scalar.activation(x0T_f[:, i:i + w], ps[:, :w],
                                     mybir.ActivationFunctionType.Relu)
                nc.vector.tensor_copy(x0T_bf[:, i:i + w], x0T_f[:, i:i + w])
            # m0 node-major -> m_shard
            for mb in range(nmacro):
                sl = slice(mb * 128, (mb + 1) * 128)
                ps_full = prepp.tile([128, 512], F32, tag="pre")
                ps = ps_full[:, 0:128]
                nc.tensor.matmul(out=ps[:], lhsT=xT_sb[:, sl], rhs=lwT_sb[:],
                                 start=True, stop=True)
                mb_sb = sb.tile([128, 128], BF16, tag="m0s")
                nc.scalar.activation(mb_sb[:], ps[:],
                                     mybir.ActivationFunctionType.Relu)
                nc.sync.dma_start(m_shard[mb * 128:(mb + 1) * 128, :],
                                  mb_sb[:])

            for step in range(STEPS):
                nc.gpsimd.collective_compute(
                    "AllGather", mybir.AluOpType.bypass,
                    replica_groups=[list(range(NCORES))],
                    ins=[m_shard.ap().opt()], outs=[m_table.ap().opt()])

                for g in range(ngroups):
                    mg = mgp.tile([128, 64, 128], BF16, tag="mg")
                    nc.gpsimd.dma_gather(
                        out_ap=mg[:, 0:32, :], in_ap=m_table[:, :],
                        idxs_ap=gidx_sb[:, g * 512:g * 512 + 256],
                        num_idxs=4096, num_idxs_reg=nidx_reg, elem_size=128,
                        single_packet=False)
                    nc.gpsimd.dma_gather(
                        out_ap=mg[:, 32:64, :], in_ap=m_table[B_BASE:, :],
                        idxs_ap=gidx_sb[:, g * 512 + 256:g * 512 + 512],
                        num_idxs=4096, num_idxs_reg=nidx_reg, elem_size=128,
                        single_packet=False)
                    oh = ohp.tile([128, 64, 128], BF16, tag="oh")
                    dsl = dstl_sb[:, g * 64:(g + 1) * 64]
                    nc.vector.tensor_tensor(
                        out=oh[:, :, :],
                        in0=dsl[:, :, None].to_broadcast([128, 64, 128]),
                        in1=iota[:, None, :].to_broadcast([128, 64, 128]),
                        op=mybir.AluOpType.is_equal)

                    for half in range(4):      # four macro-blocks per group
                        mb = g * 4 + half
                        aggT = aggp.tile([128, 128], F32, tag="agg")
                        for sl4 in range(4):   # sub-blocks in macro
                            s_loc = half * 4 + sl4
                            pt = ptp.tile([128, 128], F32, tag="pt")
                            for k in range(4):
                                ch = (2 * s_loc + k % 2) + (32 if k >= 2 else 0)
                                nc.tensor.matmul(
                                    out=pt[:], lhsT=mg[:, ch, :],
                                    rhs=oh[:, ch, :],
                                    start=(k == 0), stop=(k == 3))
                            pt_sb = sb.tile([128, 128], BF16, tag="pts")
                            nc.scalar.activation(
                                pt_sb[:], pt[:],
                                mybir.ActivationFunctionType.Copy)
                            cs = slice(sl4 * 32, (sl4 + 1) * 32)
                            for t in range(T):
                                nc.tensor.matmul(
                                    out=aggT[:, cs], lhsT=WtT2(step, t),
                                    rhs=pt_sb[:, t * 32:(t + 1) * 32],
                                    start=(t == 0), stop=(t == 3))
                        agg_sb = sb.tile([128, 128], BF16, tag="aggs")
                        nc.scalar.activation(agg_sb[:], aggT[:],
                                             mybir.ActivationFunctionType.Copy)

                        msl = slice(mb * 128, (mb + 1) * 128)
                        GT = gatep.tile([128, 512], F32, tag="GT")
                        for gi, (wT, rhs, lone) in enumerate((
                                (wihT[:, 0:128], agg_sb, False),
                                (wihT[:, 128:256], agg_sb, False),
                                (wihT[:, 256:384], agg_sb, True),
                                (whhT[:, 256:384], None, True))):
                            out_sl = GT[:, gi * 128:(gi + 1) * 128]
                            if gi < 3:
                                nc.tensor.matmul(out=out_sl, lhsT=wT,
                                                 rhs=rhs[:], start=True,
                                                 stop=lone)
                                if not lone:
                                    nc.tensor.matmul(
                                        out=out_sl,
                                        lhsT=whhT[:, gi * 128:(gi + 1) * 128],
                                        rhs=x0T_bf[:, msl], start=False,
                                        stop=True)
                            else:
                                nc.tensor.matmul(out=out_sl, lhsT=wT,
                                                 rhs=x0T_bf[:, msl],
                                                 start=True, stop=True)
                        r_sb = sb.tile([128, 128], BF16, tag="r")
                        nc.scalar.activation(
                            r_sb[:], GT[:, 0:128],
                            mybir.ActivationFunctionType.Sigmoid,
                            bias=bia[:, 0:1])
                        z_sb = sb.tile([128, 128], BF16, tag="z")
                        nc.scalar.activation(
                            z_sb[:], GT[:, 128:256],
                            mybir.ActivationFunctionType.Sigmoid,
                            bias=bia[:, 1:2])
                        ghn_sb = sb.tile([128, 128], F32, tag="ghn")
                        nc.scalar.activation(
                            ghn_sb[:], GT[:, 384:512],
                            mybir.ActivationFunctionType.Identity,
                            bias=bia[:, 3:4])
                        t1 = sb.tile([128, 128], F32, tag="t1")
                        nc.vector.tensor_tensor(out=t1[:], in0=r_sb[:],
                                                in1=ghn_sb[:],
                                                op=mybir.AluOpType.mult)
                        t2 = sb.tile([128, 128], F32, tag="t2")
                        nc.vector.tensor_tensor(out=t2[:], in0=t1[:],
                                                in1=GT[:, 256:384],
                                                op=mybir.AluOpType.add)
                        n_sb = sb.tile([128, 128], F32, tag="n")
                        nc.scalar.activation(
                            n_sb[:], t2[:],
                            mybir.ActivationFunctionType.Tanh,
                            bias=bia[:, 2:3])
                        d1 = sb.tile([128, 128], F32, tag="d1")
                        nc.vector.tensor_tensor(out=d1[:],
                                                in0=x0T_f[:, msl],
                                                in1=n_sb[:],
                                                op=mybir.AluOpType.subtract)
                        d2 = sb.tile([128, 128], F32, tag="d2")
                        nc.vector.tensor_tensor(out=d2[:], in0=z_sb[:],
                                                in1=d1[:],
                                                op=mybir.AluOpType.mult)
                        d3 = sb.tile([128, 128], F32, tag="d3")
                        nc.vector.tensor_tensor(out=d3[:], in0=n_sb[:],
                                                in1=d2[:],
                                                op=mybir.AluOpType.add)
                        if step < STEPS - 1:
                            mT_bf = sb.tile([128, 128], BF16, tag="mT")
                            nc.scalar.activation(
                                mT_bf[:], d3[:],
                                mybir.ActivationFunctionType.Relu)
                            tp = miscp.tile([128, 128], BF16, tag="lp")
                            nc.tensor.transpose(out=tp[:], in_=mT_bf[:],
                                                identity=ident)
                            m_sb = sb.tile([128, 128], BF16, tag="ms")
                            nc.vector.tensor_copy(m_sb[:], tp[:])
                            nc.sync.dma_start(
                                m_shard[mb * 128:(mb + 1) * 128, :], m_sb[:])
                        else:
                            mT_f = sb.tile([128, 128], F32, tag="mTf")
                            nc.scalar.activation(
                                mT_f[:], d3[:],
                                mybir.ActivationFunctionType.Relu)
                            for oi, (wsl, bsl, out_t) in enumerate((
                                    (mulv_sb[:, 0:128], bia[:, 4:5], muo),
                                    (mulv_sb[:, 128:256], bia[:, 5:6], lvo))):
                                ps = miscp.tile([128, 128], F32, tag="lp2")
                                nc.tensor.matmul(out=ps[:], lhsT=wsl,
                                                 rhs=mT_f[:], start=True,
                                                 stop=True)
                                o_sb = sb.tile([128, 128], F32, tag="osb")
                                nc.scalar.activation(
                                    o_sb[:], ps[:],
                                    mybir.ActivationFunctionType.Identity,
                                    bias=bsl)
                                nc.sync.dma_start(out_t[:, msl], o_sb[:])
    return nc


_CACHE = {}


def kernel(**inputs):
    _install_ntff_hook()
    pp = _preprocess(inputs["edge_index"], inputs["edge_attr"])
    nsub, spc, nslots = pp["nsub"], pp["spc"], pp["nslots"]
    node_slot = pp["node_slot"]

    x = np.asarray(inputs["x"], np.float32)
    lin_w = np.asarray(inputs["lin_w"], np.float32)
    lin_b = np.asarray(inputs["lin_b"], np.float32)
    gnn_w = np.asarray(inputs["gnn_w"], np.float32)
    w_ih = np.asarray(inputs["w_ih"], np.float32)
    w_hh = np.asarray(inputs["w_hh"], np.float32)
    b_ih = np.asarray(inputs["b_ih"], np.float32)
    b_hh = np.asarray(inputs["b_hh"], np.float32)
    mu_w = np.asarray(inputs["mu_w"], np.float32)
    mu_b = np.asarray(inputs["mu_b"], np.float32)
    lv_w = np.asarray(inputs["lv_w"], np.float32)
    lv_b = np.asarray(inputs["lv_b"], np.float32)

    # slotted x^T with ones row (bias via augmented matmul)
    x_slot = np.zeros((nslots, IN), np.float32)
    x_slot[node_slot] = x
    lwT_aug = np.concatenate([lin_w.T, lin_b[None, :]], 0).astype(np.float32)

    iota_t = np.tile(np.arange(128, dtype=np.float32)[None, :], (128, 1))
    ident = np.eye(128, dtype=np.float32)
    consts = np.concatenate([
        iota_t, ident, w_ih.T, w_hh.T,
        np.concatenate([(2.0 * gnn_w[s, t]).T for s in range(STEPS)
                        for t in range(T)], axis=1),
    ], axis=1).astype(BF)
    mulvT = np.concatenate([mu_w.T, lv_w.T], 1).astype(np.float32)
    biases = np.zeros((128, 8), np.float32)
    biases[:, 0] = b_ih[0:128] + b_hh[0:128]
    biases[:, 1] = b_ih[128:256] + b_hh[128:256]
    biases[:, 2] = b_ih[256:384]
    biases[:, 3] = b_hh[256:384]
    biases[:, 4] = mu_b
    biases[:, 5] = lv_b

    in_maps = []
    for c in range(NCORES):
        xs = x_slot[c * spc:(c + 1) * spc]
        xT_aug = np.concatenate([xs.T, np.ones((1, spc), np.float32)], 0)
        in_maps.append(dict(
            xT=xT_aug.astype(np.float32), lwT=lwT_aug,
            gidx=pp["gidx"][c], dstl=pp["dstl"][c].astype(BF),
            consts=consts, mulvT=mulvT, biases=biases))

    key = (nsub, spc, nslots)
    if key not in _CACHE:
        nc = _build(nsub, spc, nslots)
        mybir.codegen_inst_isa_subclasses(nc)
        _fix_sync_waits(nc)
        _CACHE[key] = nc
    nc = _CACHE[key]

    kernel.last_in_maps = in_maps
    res = run_bass_kernel_spmd(nc, in_maps, core_ids=list(range(NCORES)))
    kernel.last_results = res

    mu = np.zeros((N, L), np.float32)
    lv = np.zeros((N, L), np.float32)
    slot_core = node_slot // spc
    slot_loc = node_slot % spc
    for c in range(NCORES):
        sel = np.flatnonzero(slot_core == c)
        mu[sel] = res.results[c]["muo"][:, slot_loc[sel]].T
        lv[sel] = res.results[c]["lvo"][:, slot_loc[sel]].T
    return mu, lv



# revision 6
# speedup vs baseline: 1.9465x; 1.9465x over previous
"""Trainium2 Bass kernel for nn_Encoder GNN message passing (8 NeuronCores).

Decomposition (dst-sharded, transposed compute):
  - nodes assigned to (core, sub-block<=32 nodes, pos) slots; each sub-block
    has <=512 in-edges, split into an A-stream (table rows < 32768) and a
    B-stream (rows >= 20480, indexed relative to 20480) of 2x128-edge chunks
    each, so indices fit dma_gather's int16.
  - per chunk: P^T[j, t*32+pos] += m_g[e, j]^T @ onehot[e, t*32+pos]
  - transform: agg^T[:, sub] = sum_t (2*W_t) @ P^T[:, t*32:(t+1)*32]
    (x2 because the reference duplicates every edge)
  - GRU (h == x0 always) computed transposed per 128-slot macro-block,
    gh recomputed per step from resident x0^T via PSUM accumulation.
  - m table [NSLOTS, 128] bf16 in DRAM, AllGather'd across cores per step.
"""
import sys
import types
import numpy as np
import ml_dtypes

import concourse.bass as bass
import concourse.mybir as mybir
import concourse.tile as tile
import concourse.tile_sem_assignment as _tsa
from concourse import library_config
from concourse.bass_utils import run_bass_kernel_spmd

# 4 SWDGE queues: dma_gather descriptor rings drain ~3x faster when cycled
_tsa.NUM_SWDGE_GLOBAL_SEMS = 4
_tsa.NUM_HWDGE_SEMS = 2

N, IN, L, STEPS, T, E = 50000, 64, 128, 3, 4, 800000
NCORES = 8
SUB_CAP_N = 32
SUB_CAP_E = 512           # 4 chunks x 128
NODES_PER_CORE = N // NCORES
B_BASE = 20480            # B-stream table base row
A_MAX = 32768             # A-stream rows must be < 32768
PAD_COL = 200.0           # one-hot sentinel (never equals iota 0..127)

F32, BF16, I16 = mybir.dt.float32, mybir.dt.bfloat16, mybir.dt.int16
BF = ml_dtypes.bfloat16


# ---------------------------------------------------------------- waitfix --
def _fix_sync_waits(nc):
    """This walrus accepts only ONE semaphore wait per instruction; move
    excess waits onto preceding same-engine NoOps (engine queues are
    in-order, so semantics are preserved)."""
    uid = [0]

    def mknop(engine, waits, debug):
        uid[0] += 1
        return mybir.InstNoOp(
            name=f"WFIX-{uid[0]}", engine=engine, ins=[], outs=[], debug=debug,
            sync_info=mybir.SyncInfo(on_wait=list(waits), on_update=[]))

    total = 0
    for bb in nc.main_func.blocks:
        il = bb.instructions
        i = 0
        while i < len(il):
            inst = il[i]
            si = inst.sync_info
            waits = list(si.on_wait) if si is not None else []
            if len(waits) > 1:
                inst.sync_info = mybir.SyncInfo(
                    on_wait=waits[:1], on_update=list(si.on_update))
                nops = [mknop(inst.engine, [w], inst.debug)
                        for w in waits[1:]]
                for k, nop in enumerate(nops):
                    il.insert(i + k, nop)
                    nc.register_instruction(nop, overwrite=True)
                i += len(nops)
                total += len(nops)
            i += 1
    return total


def _install_ntff_hook():
    if "antenv.axon_hooks" in sys.modules:
        return
    try:
        from trn_agent_boot.trn_boot import _ntff_profile_via_ctypes
        hook = _ntff_profile_via_ctypes("/opt/axon/libaxon_pjrt.so")
    except Exception:
        hook = None
    mod = types.ModuleType("antenv.axon_hooks")
    mod.get_axon_ntff_profile_hook = lambda: hook
    mod.set_axon_ntff_profile_hook = lambda h: None
    sys.modules["antenv.axon_hooks"] = mod
    import concourse.bass_utils as bu
    bu.upload_artifacts = lambda d: f"local:{d}"


# ---------------------------------------------------------- preprocessing --
def _preprocess(edge_index, edge_attr):
    src = np.asarray(edge_index[0], np.int64)
    dst = np.asarray(edge_index[1], np.int64)
    bond = np.asarray(np.argmax(np.asarray(edge_attr), axis=1), np.int64)
    degsum = np.bincount(dst, minlength=N)

    # stage 1: nodes -> cores (balance total degree, exact node count)
    order = np.argsort(-degsum, kind="stable")
    node_core = np.full(N, -1, np.int64)
    core_load = np.zeros(NCORES, np.int64)
    core_count = np.zeros(NCORES, np.int64)
    for n in order:
        cand = np.flatnonzero(core_count < NODES_PER_CORE)
        c = cand[np.argmin(core_load[cand])]
        node_core[n] = c
        core_load[c] += degsum[n]
        core_count[c] += 1

    # stage 2: per-core packing into nsub bins (<=32 nodes, <=512 edges)
    nsub = max(int(np.ceil(NODES_PER_CORE / (SUB_CAP_N - 1.5))),
               int(np.ceil(core_load.max() / (SUB_CAP_E - 16))))
    nsub = -(-nsub // 16) * 16
    for _attempt in range(6):
        packs, ok = [], True
        for c in range(NCORES):
            nodes = np.flatnonzero(node_core == c)
            ds = degsum[nodes]
            node_sub = np.full(len(nodes), -1, np.int64)
            node_pos = np.full(len(nodes), -1, np.int64)
            cnt = np.zeros(nsub, np.int64)
            load = np.zeros(nsub, np.int64)
            soft_e, soft_n = SUB_CAP_E - 12, SUB_CAP_N - 1
            for i in np.argsort(-ds, kind="stable"):
                l2, c2 = load + ds[i], cnt + 1
                hard = (c2 > SUB_CAP_N) | (l2 > SUB_CAP_E)
                key = np.where(hard, 1e18,
                               np.maximum(l2 / soft_e, c2 / soft_n))
                b = int(np.argmin(key))
                if key[b] >= 1e17:
                    ok = False
                    break
                node_sub[i] = b
                node_pos[i] = cnt[b]
                cnt[b] += 1
                load[b] += ds[i]
            if not ok:
                break
            packs.append((nodes, node_sub, node_pos))
        if ok:
            break
        nsub += 16
    assert ok, "node packing failed"

    spc = nsub * 32                       # slots per core
    nslots = NCORES * spc
    node_slot = np.full(N, -1, np.int64)
    for c, (nodes, nsubv, nposv) in enumerate(packs):
        node_slot[nodes] = c * spc + nsubv * 32 + nposv

    # per-edge info
    e_core = node_core[dst]
    e_sub = (node_slot[dst] % spc) // 32
    e_col = bond * 32 + (node_slot[dst] % 32)
    e_srcslot = node_slot[src]

    # per-core chunk arrays in the group layout:
    #   group g = subs [16g, 16g+16); tile cols 0..31 = A chunks (2/sub),
    #   32..63 = B chunks (2/sub)
    ngroups = nsub // 16
    nchunks = nsub * 4
    # idx tensor: [128, ngroups*512] int16 (per group: 256 A cols + 256 B)
    gidx = np.zeros((NCORES, 128, ngroups * 512), np.int16)
    dstl = np.full((NCORES, 128, nchunks), PAD_COL, np.float32)

    for c in range(NCORES):
        esel = np.flatnonzero(e_core == c)
        sub = e_sub[esel]
        order_e = np.argsort(sub, kind="stable")
        es, subs_sorted = esel[order_e], sub[order_e]
        starts = np.searchsorted(subs_sorted, np.arange(nsub))
        ends = np.searchsorted(subs_sorted, np.arange(nsub), side="right")
        # linear index lists per group: A_lin[g][i] = table row, i -> (chunk
        # i//128 within the A half, partition i%128)
        A_lin = np.zeros((ngroups, 4096), np.int64)
        B_lin = np.zeros((ngroups, 4096), np.int64)
        for s in range(nsub):
            e_seg = es[starts[s]:ends[s]]
            slots = e_srcslot[e_seg]
            isA_must = slots < B_BASE
            isB_must = slots >= A_MAX
            band = ~(isA_must | isB_must)
            a_cnt = int(isA_must.sum())
            take_band_a = min(max(0, 256 - a_cnt), int(band.sum()))
            band_idx = np.flatnonzero(band)
            a_sel = np.concatenate(
                [np.flatnonzero(isA_must), band_idx[:take_band_a]])
            b_sel = np.concatenate(
                [np.flatnonzero(isB_must), band_idx[take_band_a:]])
            assert len(a_sel) <= 256 and len(b_sel) <= 256, \
                f"A/B split overflow sub {s}: {len(a_sel)} {len(b_sel)}"
            g, sl = s // 16, s % 16
            for sel, lin, base in ((a_sel, A_lin, 0), (b_sel, B_lin, B_BASE)):
                rows = slots[sel] - base
                cols = e_col[e_seg[sel]]
                k = np.arange(len(sel))
                lin[g, sl * 256 + k] = rows
                ch_off = (0 if base == 0 else 32) + 2 * sl
                dstl[c, k % 128, (g * 64 + ch_off + k // 128)] = cols
        # wrap indices: position i -> (partition i%16? no: (s p) wrap over 16)
        for g in range(ngroups):
            for half, lin in ((0, A_lin), (1, B_lin)):
                w = lin[g].reshape(256, 16).T        # [16, 256]
                gidx[c, :, g * 512 + half * 256:(g * 512 + half * 256) + 256] \
                    = np.tile(w, (8, 1))
    return dict(node_slot=node_slot, nsub=nsub, spc=spc, nslots=nslots,
                gidx=gidx, dstl=dstl)


# ------------------------------------------------------------- bass graph --
def _build(nsub, spc, nslots):
    nc = bass.Bass(target_bir_lowering=False, debug=False,
                   dynamic_dma_scratch_size=65536, num_swdge_queues=4)
    ngroups = nsub // 16
    nmacro = nsub // 4
    nchunks = nsub * 4

    xT = nc.declare_dram_parameter("xT", [IN + 1, spc], F32, isOutput=False)
    lwT = nc.declare_dram_parameter("lwT", [IN + 1, 128], F32, isOutput=False)
    gidx = nc.declare_dram_parameter("gidx", [128, ngroups * 512], I16,
                                     isOutput=False)
    dstl = nc.declare_dram_parameter("dstl", [128, nchunks], BF16,
                                     isOutput=False)
    # consts bf16: iota(128) | identity(128) | w_ihT(384) | w_hhT(384) |
    #              WtT2 (12*128)
    CCOLS = 128 + 128 + 384 + 384 + 12 * 128
    consts = nc.declare_dram_parameter("consts", [128, CCOLS], BF16,
                                       isOutput=False)
    mulvT = nc.declare_dram_parameter("mulvT", [128, 256], F32, isOutput=False)
    biases = nc.declare_dram_parameter("biases", [128, 8], F32, isOutput=False)
    muo = nc.declare_dram_parameter("muo", [128, spc], F32, isOutput=True)
    lvo = nc.declare_dram_parameter("lvo", [128, spc], F32, isOutput=True)

    m_shard = nc.dram_tensor("m_shard", [spc, 128], BF16, kind="Internal")
    m_table = nc.dram_tensor("m_table", [nslots, 128], BF16, kind="Internal",
                             addr_space="Shared")

    with tile.TileContext(nc) as tc:
        with (
            tc.tile_pool(name="const", bufs=1) as cpool,
            tc.tile_pool(name="sb", bufs=2) as sb,
            tc.tile_pool(name="mg", bufs=2) as mgp,
            tc.tile_pool(name="oh", bufs=2) as ohp,
            tc.tile_pool(name="pt", bufs=2, space="PSUM") as ptp,
            tc.tile_pool(name="agg", bufs=1, space="PSUM") as aggp,
            tc.tile_pool(name="gate", bufs=1, space="PSUM") as gatep,
            tc.tile_pool(name="misc", bufs=1, space="PSUM") as miscp,
            tc.tile_pool(name="prep", bufs=2, space="PSUM") as prepp,
        ):
            nc.gpsimd.load_library(library_config.mlp)
            nidx_reg = nc.gpsimd.to_reg(2048)

            cst = cpool.tile([128, CCOLS], BF16)
            nc.sync.dma_start(cst[:], consts[:, :])
            iota = cst[:, 0:128]
            ident = cst[:, 128:256]
            wihT = cst[:, 256:640]
            whhT = cst[:, 640:1024]

            def WtT2(step, t):
                o = 1024 + (step * 4 + t) * 128
                return cst[:, o:o + 128]

            mulv_sb = cpool.tile([128, 256], F32)
            nc.sync.dma_start(mulv_sb[:], mulvT[:, :])
            bia = cpool.tile([128, 8], F32)
            nc.sync.dma_start(bia[:], biases[:, :])
            gidx_sb = cpool.tile([128, ngroups * 512], I16)
            nc.sync.dma_start(gidx_sb[:], gidx[:, :])
            dstl_sb = cpool.tile([128, nchunks], BF16)
            nc.sync.dma_start(dstl_sb[:], dstl[:, :])
            xT_sb = cpool.tile([IN + 1, spc], F32)
            nc.sync.dma_start(xT_sb[:], xT[:, :])
            lwT_sb = cpool.tile([IN + 1, 128], F32)
            nc.sync.dma_start(lwT_sb[:], lwT[:, :])

            # ---- x0 ----
            x0T_bf = cpool.tile([128, spc], BF16)
            for i in range(0, spc, 512):
                w = min(512, spc - i)
                ps = prepp.tile([128, 512], F32, tag="pre")
                nc.tensor.matmul(out=ps[:, :w], lhsT=lwT_sb[:],
                                 rhs=xT_sb[:, i:i + w], start=True, stop=True)
                nc.scalar.activation(x0T_bf[:, i:i + w], ps[:, :w],
                                     mybir.ActivationFunctionType.Relu)
            # m0 node-major -> m_shard
            for mb in range(nmacro):
                sl = slice(mb * 128, (mb + 1) * 128)
                ps_full = prepp.tile([128, 512], F32, tag="pre")
                ps = ps_full[:, 0:128]
                nc.tensor.matmul(out=ps[:], lhsT=xT_sb[:, sl], rhs=lwT_sb[:],
                                 start=True, stop=True)
                mb_sb = sb.tile([128, 128], BF16, tag="m0s")
                nc.scalar.activation(mb_sb[:], ps[:],
                                     mybir.ActivationFunctionType.Relu)
                nc.sync.dma_start(m_shard[mb * 128:(mb + 1) * 128, :],
                                  mb_sb[:])

            for step in range(STEPS):
                nc.gpsimd.collective_compute(
                    "AllGather", mybir.AluOpType.bypass,
                    replica_groups=[list(range(NCORES))],
                    ins=[m_shard.ap().opt()], outs=[m_table.ap().opt()])

                for g in range(ngroups):
                    mg = mgp.tile([128, 64, 128], BF16, tag="mg")
                    for q in range(4):
                        half_base = 0 if q < 2 else 32
                        src_t = m_table[:, :] if q < 2 else m_table[B_BASE:, :]
                        co = g * 512 + (q % 2) * 128 + (0 if q < 2 else 256)
                        nc.gpsimd.dma_gather(
                            out_ap=mg[:, half_base + (q % 2) * 16:
                                      half_base + (q % 2) * 16 + 16, :],
                            in_ap=src_t,
                            idxs_ap=gidx_sb[:, co:co + 128],
                            num_idxs=2048, num_idxs_reg=nidx_reg,
                            elem_size=128, single_packet=False, queue_num=q)
                    oh = ohp.tile([128, 64, 128], BF16, tag="oh")
                    dsl = dstl_sb[:, g * 64:(g + 1) * 64]
                    nc.vector.tensor_tensor(
                        out=oh[:, :, :],
                        in0=dsl[:, :, None].to_broadcast([128, 64, 128]),
                        in1=iota[:, None, :].to_broadcast([128, 64, 128]),
                        op=mybir.AluOpType.is_equal)

                    for half in range(4):      # four macro-blocks per group
                        mb = g * 4 + half
                        ptm = sb.tile([128, 4, 128], BF16, tag="ptm")
                        for sl4 in range(4):   # sub-blocks in macro
                            s_loc = half * 4 + sl4
                            pt = ptp.tile([128, 128], F32, tag="pt")
                            for k in range(4):
                                ch = (2 * s_loc + k % 2) + (32 if k >= 2 else 0)
                                nc.tensor.matmul(
                                    out=pt[:], lhsT=mg[:, ch, :],
                                    rhs=oh[:, ch, :],
                                    start=(k == 0), stop=(k == 3))
                            if sl4 % 2 == 0:
                                nc.scalar.activation(
                                    ptm[:, sl4, :], pt[:],
                                    mybir.ActivationFunctionType.Copy)
                            else:
                                nc.vector.tensor_copy(ptm[:, sl4, :], pt[:])
                        aggT = aggp.tile([128, 128], F32, tag="agg")
                        for t in range(T):
                            nc.tensor.matmul(
                                out=aggT[:], lhsT=WtT2(step, t),
                                rhs=ptm[:, :, t * 32:(t + 1) * 32],
                                start=(t == 0), stop=(t == 3))
                        agg_sb = sb.tile([128, 128], BF16, tag="aggs")
                        nc.scalar.activation(agg_sb[:], aggT[:],
                                             mybir.ActivationFunctionType.Copy)

                        msl = slice(mb * 128, (mb + 1) * 128)
                        GT = gatep.tile([128, 512], F32, tag="GT")
                        for gi, (wT, rhs, lone) in enumerate((
                                (wihT[:, 0:128], agg_sb, False),
                                (wihT[:, 128:256], agg_sb, False),
                                (wihT[:, 256:384], agg_sb, True),
                                (whhT[:, 256:384], None, True))):
                            out_sl = GT[:, gi * 128:(gi + 1) * 128]
                            if gi < 3:
                                nc.tensor.matmul(out=out_sl, lhsT=wT,
                                                 rhs=rhs[:], start=True,
                                                 stop=lone)
                                if not lone:
                                    nc.tensor.matmul(
                                        out=out_sl,
                                        lhsT=whhT[:, gi * 128:(gi + 1) * 128],
                                        rhs=x0T_bf[:, msl], start=False,
                                        stop=True)
                            else:
                                nc.tensor.matmul(out=out_sl, lhsT=wT,
                                                 rhs=x0T_bf[:, msl],
                                                 start=True, stop=True)
                        r_sb = sb.tile([128, 128], BF16, tag="r")
                        nc.scalar.activation(
                            r_sb[:], GT[:, 0:128],
                            mybir.ActivationFunctionType.Sigmoid,
                            bias=bia[:, 0:1])
                        z_sb = sb.tile([128, 128], BF16, tag="z")
                        nc.scalar.activation(
                            z_sb[:], GT[:, 128:256],
                            mybir.ActivationFunctionType.Sigmoid,
                            bias=bia[:, 1:2])
                        ghn_sb = sb.tile([128, 128], F32, tag="ghn")
                        nc.scalar.activation(
                            ghn_sb[:], GT[:, 384:512],
                            mybir.ActivationFunctionType.Identity,
                            bias=bia[:, 3:4])
                        t1 = sb.tile([128, 128], F32, tag="t1")
                        nc.vector.tensor_tensor(out=t1[:], in0=r_sb[:],
                                                in1=ghn_sb[:],
                                                op=mybir.AluOpType.mult)
                        t2 = sb.tile([128, 128], F32, tag="t2")
                        nc.vector.tensor_tensor(out=t2[:], in0=t1[:],
                                                in1=GT[:, 256:384],
                                                op=mybir.AluOpType.add)
                        n_sb = sb.tile([128, 128], F32, tag="n")
                        nc.scalar.activation(
                            n_sb[:], t2[:],
                            mybir.ActivationFunctionType.Tanh,
                            bias=bia[:, 2:3])
                        d1 = sb.tile([128, 128], F32, tag="d1")
                        nc.vector.tensor_tensor(out=d1[:],
                                                in0=x0T_bf[:, msl],
                                                in1=n_sb[:],
                                                op=mybir.AluOpType.subtract)
                        d2 = sb.tile([128, 128], F32, tag="d2")
                        nc.vector.tensor_tensor(out=d2[:], in0=z_sb[:],
                                                in1=d1[:],
                                                op=mybir.AluOpType.mult)
                        d3 = sb.tile([128, 128], F32, tag="d3")
                        nc.vector.tensor_tensor(out=d3[:], in0=n_sb[:],
                                                in1=d2[:],
                                                op=mybir.AluOpType.add)
                        if step < STEPS - 1:
                            mT_bf = sb.tile([128, 128], BF16, tag="mT")
                            nc.scalar.activation(
                                mT_bf[:], d3[:],
                                mybir.ActivationFunctionType.Relu)
                            tp = miscp.tile([128, 128], BF16, tag="lp")
                            nc.tensor.transpose(out=tp[:], in_=mT_bf[:],
                                                identity=ident)
                            m_sb = sb.tile([128, 128], BF16, tag="ms")
                            nc.vector.tensor_copy(m_sb[:], tp[:])
                            nc.sync.dma_start(
                                m_shard[mb * 128:(mb + 1) * 128, :], m_sb[:])
                        else:
                            mT_f = sb.tile([128, 128], F32, tag="mTf")
                            nc.scalar.activation(
                                mT_f[:], d3[:],
                                mybir.ActivationFunctionType.Relu)
                            for oi, (wsl, bsl, out_t) in enumerate((
                                    (mulv_sb[:, 0:128], bia[:, 4:5], muo),
                                    (mulv_sb[:, 128:256], bia[:, 5:6], lvo))):
                                ps = miscp.tile([128, 128], F32, tag="lp2")
                                nc.tensor.matmul(out=ps[:], lhsT=wsl,
                                                 rhs=mT_f[:], start=True,
                                                 stop=True)
                                o_sb = sb.tile([128, 128], F32, tag="osb")
                                nc.scalar.activation(
                                    o_sb[:], ps[:],
                                    mybir.ActivationFunctionType.Identity,
                                    bias=bsl)
                                nc.sync.dma_start(out_t[:, msl], o_sb[:])
    return nc


_CACHE = {}


def kernel(**inputs):
    _install_ntff_hook()
    pp = _preprocess(inputs["edge_index"], inputs["edge_attr"])
    nsub, spc, nslots = pp["nsub"], pp["spc"], pp["nslots"]
    node_slot = pp["node_slot"]

    x = np.asarray(inputs["x"], np.float32)
    lin_w = np.asarray(inputs["lin_w"], np.float32)
    lin_b = np.asarray(inputs["lin_b"], np.float32)
    gnn_w = np.asarray(inputs["gnn_w"], np.float32)
    w_ih = np.asarray(inputs["w_ih"], np.float32)
    w_hh = np.asarray(inputs["w_hh"], np.float32)
    b_ih = np.asarray(inputs["b_ih"], np.float32)
    b_hh = np.asarray(inputs["b_hh"], np.float32)
    mu_w = np.asarray(inputs["mu_w"], np.float32)
    mu_b = np.asarray(inputs["mu_b"], np.float32)
    lv_w = np.asarray(inputs["lv_w"], np.float32)
    lv_b = np.asarray(inputs["lv_b"], np.float32)

    # slotted x^T with ones row (bias via augmented matmul)
    x_slot = np.zeros((nslots, IN), np.float32)
    x_slot[node_slot] = x
    lwT_aug = np.concatenate([lin_w.T, lin_b[None, :]], 0).astype(np.float32)

    iota_t = np.tile(np.arange(128, dtype=np.float32)[None, :], (128, 1))
    ident = np.eye(128, dtype=np.float32)
    consts = np.concatenate([
        iota_t, ident, w_ih.T, w_hh.T,
        np.concatenate([(2.0 * gnn_w[s, t]).T for s in range(STEPS)
                        for t in range(T)], axis=1),
    ], axis=1).astype(BF)
    mulvT = np.concatenate([mu_w.T, lv_w.T], 1).astype(np.float32)
    biases = np.zeros((128, 8), np.float32)
    biases[:, 0] = b_ih[0:128] + b_hh[0:128]
    biases[:, 1] = b_ih[128:256] + b_hh[128:256]
    biases[:, 2] = b_ih[256:384]
    biases[:, 3] = b_hh[256:384]
    biases[:, 4] = mu_b
    biases[:, 5] = lv_b

    in_maps = []
    for c in range(NCORES):
        xs = x_slot[c * spc:(c + 1) * spc]
        xT_aug = np.concatenate([xs.T, np.ones((1, spc), np.float32)], 0)
        in_maps.append(dict(
            xT=xT_aug.astype(np.float32), lwT=lwT_aug,
            gidx=pp["gidx"][c], dstl=pp["dstl"][c].astype(BF),
            consts=consts, mulvT=mulvT, biases=biases))

    key = (nsub, spc, nslots)
    if key not in _CACHE:
        nc = _build(nsub, spc, nslots)
        mybir.codegen_inst_isa_subclasses(nc)
        _fix_sync_waits(nc)
        _CACHE[key] = nc
    nc = _CACHE[key]

    kernel.last_in_maps = in_maps
    res = run_bass_kernel_spmd(nc, in_maps, core_ids=list(range(NCORES)))
    kernel.last_results = res

    mu = np.zeros((N, L), np.float32)
    lv = np.zeros((N, L), np.float32)
    slot_core = node_slot // spc
    slot_loc = node_slot % spc
    for c in range(NCORES):
        sel = np.flatnonzero(slot_core == c)
        mu[sel] = res.results[c]["muo"][:, slot_loc[sel]].T
        lv[sel] = res.results[c]["lvo"][:, slot_loc[sel]].T
    return mu, lv

